# revision 9
# baseline (speedup 1.0000x reference)
"""Trainium2 Bass kernel for the cross-batch retrieval contrastive loss.

Pipeline per batch b (reference semantics):
  sent_mean = mean(sent_feat * masks)                     (host)
  v1   = conv1([bef^T; broadcast sent_mean])              -> (196, 512)
  MHA over 196 positions, out_proj                        -> (196, 512)
  mod  = conv2(o)                                         -> (196, 512)
  ql   = mod @ q_w^T + q_b ; kl = aft @ k_w^T + k_b       -> (196, 512)
  logits[a,b,l,m] = ql[a,l,:] . kl[b,m,:]   (head split is a no-op)
  t2v[a,b] = mean_l max_m ; v2t[a,b] = mean_m max_l
  loss = symmetric InfoNCE on S = 0.5*(t2v+v2t)*exp(logit_scale)  (host, 32x32)

Sharding: data-parallel over the query-batch axis 'a' (4 batches/core x 8
cores). Each core computes kl for all 32 key batches (replicated), its own
front-end, and both orientations of every (a, b) logits tile so that the
max over l and the max over m are both free-axis DVE reductions.

Performance structure: the kernel is DVE-bound (the 210 max-reductions over
fp32 PSUM run at 1 elem/cycle/lane with a 120-cycle PSUM access penalty per
instruction; no DVE fast mode applies to PSUM reads).  Design choices:
- every linear chain with no nonlinearity is folded on the host:
  q/k/v come straight from bef (per-batch text term as bias), and
  out_proj -> conv2 -> q_w is ONE matrix; weights pre-scaled so fp8
  entries sit in the format's comfortable range, with the inverse scale
  applied on the PSUM->SBUF ACT copy;
- logits matmuls write 2-bank PSUM tiles [128, 2, 512] so each DVE
  max-reduce covers 4 b-groups ([qn, 2, 2, 196] -> [qn, 4]), amortizing
  the PSUM access penalty;
- a tunable fraction of windows is instead copied by ACT to bf16 SBUF
  and scanned by DVE tensor_scalar accum-max at 4x (111ns/196 vs 1x from
  PSUM), balancing ACT vs DVE occupancy;
- softmax: all 8 heads' exp-colsums land in one [8, 196] psum tile (one
  matmul per (head, m-tile) with a row-offset output), ONE reciprocal per
  batch covers all heads, and a [2,128]-selector matmul broadcasts the
  per-head normalizers to the 128 partitions of each head pair;
- final 32x32 InfoNCE on the host in float64 (tiny).
"""

import numpy as np
import ml_dtypes

B, LV, LT, D, H = 32, 196, 40, 512, 8
NCORES = 8
AL = B // NCORES          # query batches per core
KT = D // 128             # 128-row feature tiles per 512-dim tensor
LSPLIT = [(0, 128), (128, 68)]   # 196 = 128 + 68
NQ = AL * LV              # 784 query position-rows per core
NKEY = B * LV             # 6272 key position-rows
TQ = (NQ + 127) // 128    # 7 stationary tiles over query rows
TK = NKEY // 128          # 49 stationary tiles over key rows
W2 = 2 * LV               # batch-pair moving width
W2P = 400                 # fe fp8 tile stride (16B-aligned for DoubleRow)
BF16 = ml_dtypes.bfloat16
F8 = ml_dtypes.float8_e4m3fn

# fp8 scale plan:
#   bef, q, k: x1   v(pos): x1   ot: x32 (via the 1/32 colsum)
#   ql: x128  kl: x8
# Combined weights are additionally pre-scaled by SW_* on the host so the
# fp8 entries are O(0.1); the ACT copy applies 1/SW_*.
FP8_SQ = 128.0
FP8_SK = 8.0
S_OT = 32.0
SW_QKV = 16.0             # host pre-scale on wq8/wk8/wv8 (copy scale 1/16)
SW_QL = 8.0               # extra pre-scale on the combined ql weight

# fraction control for ACT-staged windows: stage every STAGE_MOD-th
# double-window group (0 disables staging)
STAGE_O1 = 2  # stage o1 group when (group_idx % STAGE_O1) == 0
STAGE_O2 = 0  # o2 staging off initially

_CACHE = {}


def _build_program(reps=1):
    from contextlib import ExitStack
    import concourse.bacc as bacc
    import concourse.tile as tile
    from concourse import mybir

    f32 = mybir.dt.float32
    bf = mybir.dt.bfloat16
    f8 = mybir.dt.float8e4

    nc = bacc.Bacc("TRN2", target_bir_lowering=False, debug=False,
                   num_devices=NCORES)

    d = {
        "befT": nc.dram_tensor("befT", [128, KT, NQ], f8,
                               kind="ExternalInput").ap(),
        "aftT": nc.dram_tensor("aftT", [128, KT, NKEY], f8,
                               kind="ExternalInput").ap(),
        # per-batch per-partition bias terms (true units): q/k text bias
        "tq": nc.dram_tensor("tq", [128, KT * AL], f32,
                             kind="ExternalInput").ap(),
        "tk": nc.dram_tensor("tk", [128, KT * AL], f32,
                             kind="ExternalInput").ap(),
        # v text contribution x16, row-major on one partition: [1, AL, D]
        "tv16": nc.dram_tensor("tv16", [1, AL, D], bf,
                               kind="ExternalInput").ap(),
        "bqlc": nc.dram_tensor("bqlc", [128, KT], f32,
                               kind="ExternalInput").ap(),
        "bkl": nc.dram_tensor("bkl", [128, KT], f32,
                              kind="ExternalInput").ap(),
        "amask": nc.dram_tensor("amask", [128, TQ * AL], bf,
                                kind="ExternalInput").ap(),
        "bmask": nc.dram_tensor("bmask", [128, TK * B], bf,
                                kind="ExternalInput").ap(),
        "out": nc.dram_tensor("out", [2 * AL, B], f32, kind="ExternalOutput").ap(),
    }
    for n in ["wq8", "wk8", "wv8", "wql8", "wkl8"]:
        d[n] = nc.dram_tensor(n, [128, KT, D], f8, kind="ExternalInput").ap()

    with tile.TileContext(nc) as tc, ExitStack() as ctx:
        const = ctx.enter_context(tc.tile_pool(name="const", bufs=1))
        big = ctx.enter_context(tc.tile_pool(name="big", bufs=1))

        for _rep in range(reps):
            _kernel_body(nc, tc, mybir, const, big, d)

    nc.compile()
    return nc


def _kernel_body(nc, tc, mybir, const, big, d):
    from contextlib import ExitStack

    f32 = mybir.dt.float32
    bf = mybir.dt.bfloat16
    f8 = mybir.dt.float8e4
    AX = mybir.AxisListType.X
    MAX = mybir.AluOpType.max
    MULT = mybir.AluOpType.mult
    EXP = mybir.ActivationFunctionType.Exp
    IDENT = mybir.ActivationFunctionType.Identity
    DR = mybir.MatmulPerfMode.DoubleRow

    # ---- constants / weights into SBUF ----
    ones = const.tile([1, 128], bf, name="ones", tag="ones")
    nc.vector.memset(ones[:], 1.0)
    ones32 = const.tile([128, 1], bf, name="ones32", tag="ones32")
    nc.vector.memset(ones32[:], 1.0 / S_OT)

    sb_in = {}
    for n, dt_ in [("tq", f32), ("tk", f32), ("tv16", bf), ("bqlc", f32),
                   ("bkl", f32), ("amask", bf), ("bmask", bf)]:
        shape = list(d[n].tensor.shape)
        sb_in[n] = const.tile(shape, dt_, name=f"{n}_sb", tag=f"{n}_sb")
        nc.sync.dma_start(out=sb_in[n][:], in_=d[n][:])
    w = {}
    for n in ["wq8", "wk8", "wv8", "wql8", "wkl8"]:
        w[n] = const.tile([128, KT, D], f8, name=f"{n}_sb", tag=f"{n}_sb")
        nc.sync.dma_start(out=w[n][:], in_=d[n][:, :, :])

    befT = big.tile([128, KT, NQ], f8, name="bef8", tag="bef8")
    nc.sync.dma_start(out=befT[:], in_=d["befT"][:, :, :])
    aft = big.tile([128, KT, NKEY], f8, name="aft8", tag="aft8")
    nc.sync.dma_start(out=aft[:], in_=d["aftT"][:, :, :])
    klT = big.tile([128, KT, NKEY], f8, name="klT8", tag="klT8")
    qlT = big.tile([128, KT, NQ], f8, name="qlT8", tag="qlT8")
    cm = big.tile([128, TK, AL], bf, name="cm", tag="cm")
    rms = [big.tile([128, 4 * (NKEY // W2 // 2)], bf, name=f"rm{t}",
                    tag=f"rm{t}") for t in range(TQ)]

    # ================= front-end + klT phase =================
    with ExitStack() as fes:
        fe = fes.enter_context(tc.tile_pool(name="fe", bufs=2))
        ps = fes.enter_context(tc.tile_pool(name="ps", bufs=3, space="PSUM"))
        psb = fes.enter_context(tc.tile_pool(name="psb", bufs=3, space="PSUM"))
        ps2 = fes.enter_context(tc.tile_pool(name="ps2", bufs=2, space="PSUM"))

        def proj(dst, dst_col, src, src_col, wname, n, scale=1.0, bname=None,
                 txt_a=None, pool=None):
            """dst[:, m, dst_col:+n] (fp8) = fp8-DR W^T x src[:, :, src_col:+n],
            scale and bias applied on the PSUM->SBUF ACT copy."""
            pool = pool or ps
            for m in range(KT):
                p = pool.tile([128, 512], f32, name="p_proj", tag=pool.name)
                for j in range(KT // 2):
                    nc.tensor.matmul(
                        p[:, 0:n], lhsT=w[wname][:, 2 * j:2 * j + 2,
                                                 m * 128:(m + 1) * 128],
                        rhs=src[:, 2 * j:2 * j + 2, src_col:src_col + n],
                        start=(j == 0), stop=(j == KT // 2 - 1), perf_mode=DR)
                out_ap = dst[:, m, dst_col:dst_col + n]
                if txt_a is not None:
                    # per-batch text contribution as the copy's bias
                    for ab in range(2):
                        a = txt_a[0] + ab
                        nc.scalar.activation(
                            out_ap[:, ab * LV:(ab + 1) * LV],
                            p[:, ab * LV:(ab + 1) * LV], IDENT, scale=scale,
                            bias=sb_in[txt_a[1]][:, a * KT + m: a * KT + m + 1])
                elif bname is not None:
                    nc.scalar.activation(out_ap, p[:, 0:n], IDENT, scale=scale,
                                         bias=sb_in[bname][:, m:m + 1])
                else:
                    nc.scalar.activation(out_ap, p[:, 0:n], IDENT, scale=scale)

        # ---- front-end for the 4 local query batches (pairs) ----
        for apair in range(AL // 2):
            pc = apair * W2

            qt = fe.tile([128, KT, W2P], f8, name="qt", tag="qt")
            kt = fe.tile([128, KT, W2P], f8, name="kt", tag="kt")
            proj(qt, 0, befT, pc, "wq8", W2, scale=1.0 / SW_QKV,
                 txt_a=(apair * 2, "tq"))
            proj(kt, 0, befT, pc, "wk8", W2, scale=1.0 / SW_QKV,
                 txt_a=(apair * 2, "tk"))

            ot = fe.tile([128, KT, W2P], f8, name="ot", tag="ot")
            for ab in range(2):
                a = apair * 2 + ab
                ac = ab * LV
                # v position-major: (196, 512) as two row tiles (bf16)
                vpos = []
                for lt, (l0, ln) in enumerate(LSPLIT):
                    p5 = ps.tile([128, D], f32, name="p_vpos", tag="ps")
                    for j in range(KT // 2):
                        nc.tensor.matmul(
                            p5[0:ln, :],
                            lhsT=befT[:, 2 * j:2 * j + 2,
                                      pc + ac + l0:pc + ac + l0 + ln],
                            rhs=w["wv8"][:, 2 * j:2 * j + 2, :],
                            start=(j == 0), stop=False, perf_mode=DR)
                    # + 16x text contribution as a rank-1 update
                    nc.tensor.matmul(
                        p5[0:ln, :], lhsT=ones[0:1, 0:ln],
                        rhs=sb_in["tv16"][0:1, a, :], start=False, stop=True)
                    t = fe.tile([128, D], bf, name=f"vpos_{lt}",
                                tag=f"vpos_{lt}")
                    nc.scalar.activation(t[0:ln, :], p5[0:ln, :], IDENT,
                                         scale=1.0 / SW_QKV)
                    vpos.append(t)

                # attention; exp-scores per head, colsums for each HEAD PAIR
                # packed into one [1, 392] psum tile (both halves base-0),
                # one reciprocal per pair covers both heads
                eTs = {}
                rzbs = {}
                for kt2 in range(KT):
                    pz = ps2.tile([1, W2], f32, name="p_z", tag="ps2")
                    for hh in range(2):
                        h = kt2 * 2 + hh
                        off = 64 * hh
                        for mt, (m0, mn) in enumerate(LSPLIT):
                            psc = psb.tile([128, LV], f32, name="p_sc",
                                           tag="psb")
                            nc.tensor.matmul(
                                psc[0:mn, :],
                                lhsT=kt[off:off + 64, kt2,
                                        ac + m0:ac + m0 + mn],
                                rhs=qt[off:off + 64, kt2, ac:ac + LV],
                                start=True, stop=True)
                            e = fe.tile([128, LV], bf, name=f"eT_{h}_{mt}",
                                        tag=f"eT_{h}_{mt}")
                            nc.scalar.activation(e[0:mn, :], psc[0:mn, :],
                                                 EXP, scale=0.125)
                            eTs[(h, mt)] = e
                        for mt, (m0, mn) in enumerate(LSPLIT):
                            nc.tensor.matmul(
                                pz[0:1, hh * LV:(hh + 1) * LV],
                                lhsT=ones32[0:mn, 0:1],
                                rhs=eTs[(h, mt)][0:mn, :], start=(mt == 0),
                                stop=(mt == 1))
                    rzb = fe.tile([1, W2], bf, name=f"rzb_{kt2}",
                                  tag=f"rzb_{kt2}")
                    with nc.allow_low_precision(
                            reason="softmax normalizer consumed in fp8"):
                        nc.vector.reciprocal(rzb[:], pz[:])
                    rzbs[kt2] = rzb

                for kt2 in range(KT):
                    po = ps.tile([128, LV], f32, name="p_o", tag="ps")
                    pzb = psb.tile([128, LV], f32, name="p_zb", tag="psb")
                    for hh in range(2):
                        nc.tensor.matmul(
                            pzb[64 * hh:64 * hh + 64, :],
                            lhsT=ones[0:1, 0:64],
                            rhs=rzbs[kt2][0:1, hh * LV:(hh + 1) * LV],
                            start=True, stop=True)
                        h = kt2 * 2 + hh
                        off = 64 * hh
                        for mt, (m0, mn) in enumerate(LSPLIT):
                            nc.tensor.matmul(
                                po[off:off + 64, :],
                                lhsT=vpos[mt][0:mn, h * 64:(h + 1) * 64],
                                rhs=eTs[(h, mt)][0:mn, :], start=(mt == 0),
                                stop=(mt == 1))
                    zb = fe.tile([128, LV], f32, name="zb", tag="zb")
                    nc.scalar.copy(zb[:], pzb[:])
                    nc.vector.tensor_mul(ot[:, kt2, ac:ac + LV], po[:], zb[:])

            # combined out_proj -> conv2 -> q_w projection
            proj(qlT, pc, ot, 0, "wql8", W2, scale=1.0 / SW_QL, bname="bqlc")

        # ---- klT for all 32 key batches (fp8 DR) ----
        for w0 in range(0, NKEY, 512):
            proj(klT, w0, aft, w0, "wkl8", min(512, NKEY - w0), bname="bkl",
             pool=psb)

    # ================= logits phase: both orientations =================
    # o1: 784 query rows in 7 stationary tiles, rhs = key pairs; 2-bank psum
    # groups of 4 b's with a single [qn,2,2,196]->[qn,4] DVE reduce (or an
    # ACT-staged bf16 copy + 4x tensor_scalar accum-max scans).
    # o2: 6272 key rows in 49 stationary tiles, rhs = query pairs, same
    # grouping over pairs of stationary tiles.
    NW = NKEY // W2           # 16 windows per o1 stationary tile
    with ExitStack() as ls:
        st = ls.enter_context(tc.tile_pool(name="st", bufs=3))
        ps = ls.enter_context(tc.tile_pool(name="ps", bufs=2, space="PSUM"))
        psb = ls.enter_context(tc.tile_pool(name="psb", bufs=2, space="PSUM"))

        def scan(p, outs, staged):
            """p: [128, 2, 512] psum, 2x2 groups of 196; outs: list of 4
            [qn, 1]-ish APs (or one [qn, 4] AP as outs=[ap]) for maxes."""
            if not staged:
                nc.vector.tensor_reduce(
                    outs[0], p[:, :, 0:W2].rearrange(
                        "p k (two n) -> p k two n", two=2),
                    axis=AX, op=MAX)
            else:
                qn = p.shape[0]
                sg = st.tile([128, 2, W2], bf, name="stage", tag="st")
                nc.scalar.activation(sg[0:qn], p[:, :, 0:W2], IDENT)
                for i in range(4):
                    nc.vector.tensor_scalar(
                        out=sg[0:qn, i // 2, (i % 2) * LV:(i % 2 + 1) * LV],
                        in0=sg[0:qn, i // 2, (i % 2) * LV:(i % 2 + 1) * LV],
                        scalar1=1.0, scalar2=None, op0=MULT, op1=MAX,
                        accum_out=outs[1][i])

        def o1_stream():
            gi = 0
            for t in range(TQ):
                q0 = t * 128
                qn = min(128, NQ - q0)
                rm = rms[t]
                for g in range(NW // 2):
                    p = ps.tile([128, 2, 512], f32, name="p_lg", tag="ps")
                    for k in range(2):
                        wi = (2 * g + k) * W2
                        for j in range(KT // 2):
                            nc.tensor.matmul(
                                p[0:qn, k, 0:W2],
                                lhsT=qlT[:, 2 * j:2 * j + 2, q0:q0 + qn],
                                rhs=klT[:, 2 * j:2 * j + 2, wi:wi + W2],
                                start=(j == 0), stop=(j == KT // 2 - 1),
                                perf_mode=DR)
                    staged = STAGE_O1 and (gi % STAGE_O1 == 0)
                    scan(p[0:qn], [rm[0:qn, 4 * g:4 * g + 4],
                                   [rm[0:qn, 4 * g + i:4 * g + i + 1]
                                    for i in range(4)]], staged)
                    gi += 1
                    yield

        def o2_stream():
            gi = 0
            for apair in range(AL // 2):
                pc = apair * W2
                for g0 in range(0, TK - 1, 2):
                    p = psb.tile([128, 2, 512], f32, name="p_lg2", tag="psb")
                    for k in range(2):
                        t = g0 + k
                        for j in range(KT // 2):
                            nc.tensor.matmul(
                                p[:, k, 0:W2],
                                lhsT=klT[:, 2 * j:2 * j + 2,
                                         t * 128:(t + 1) * 128],
                                rhs=qlT[:, 2 * j:2 * j + 2, pc:pc + W2],
                                start=(j == 0), stop=(j == KT // 2 - 1),
                                perf_mode=DR)
                    staged = STAGE_O2 and (gi % STAGE_O2 == 0)
                    scan(p, [cm[:, g0:g0 + 2, 2 * apair:2 * apair + 2],
                             [cm[:, g0 + i // 2,
                                 2 * apair + i % 2:2 * apair + i % 2 + 1]
                              for i in range(4)]], staged)
                    gi += 1
                    yield
                # odd tail tile (TK = 49)
                t = TK - 1
                p = psb.tile([128, 2, 512], f32, name="p_lg2", tag="psb")
                for j in range(KT // 2):
                    nc.tensor.matmul(
                        p[:, 0, 0:W2],
                        lhsT=klT[:, 2 * j:2 * j + 2, t * 128:(t + 1) * 128],
                        rhs=qlT[:, 2 * j:2 * j + 2, pc:pc + W2],
                        start=(j == 0), stop=(j == KT // 2 - 1), perf_mode=DR)
                nc.vector.tensor_reduce(
                    cm[:, t, 2 * apair:2 * apair + 2],
                    p[:, 0, 0:W2].rearrange("p (two n) -> p two n", two=2),
                    axis=AX, op=MAX)
                yield

        s1, s2 = o1_stream(), o2_stream()
        done1 = done2 = False
        while not (done1 and done2):
            if not done1:
                done1 = next(s1, "END") == "END"
            if not done2:
                done2 = next(s2, "END") == "END"

        # ---- attribution + output ----
        OSCALE = 1.0 / (LV * FP8_SQ * FP8_SK)
        pacc = ps.tile([128, 2, 512], f32, name="pacc", tag="ps")
        for t in range(TQ):
            qn = min(128, NQ - t * 128)
            nc.tensor.matmul(pacc[0:AL, 0, 0:B],
                             lhsT=sb_in["amask"][0:qn, t * AL:(t + 1) * AL],
                             rhs=rms[t][0:qn, :],
                             start=(t == 0), stop=(t == TQ - 1))
        t2v_sb = st.tile([AL, B], f32, name="t2v_sb", tag="t2v_sb")
        nc.scalar.mul(t2v_sb[:], pacc[0:AL, 0, 0:B], OSCALE)
        nc.sync.dma_start(out=d["out"][0:AL, :], in_=t2v_sb[:])

        pv2 = psb.tile([128, 2, 512], f32, name="pv2", tag="psb")
        for t in range(TK):
            nc.tensor.matmul(pv2[0:AL, 0, 0:B], lhsT=cm[:, t, :],
                             rhs=sb_in["bmask"][:, t * B:(t + 1) * B],
                             start=(t == 0), stop=(t == TK - 1))
        v2t2 = st.tile([AL, B], f32, name="v2t2", tag="v2t2")
        nc.scalar.mul(v2t2[:], pv2[0:AL, 0, 0:B], OSCALE)
        nc.sync.dma_start(out=d["out"][AL:2 * AL, :], in_=v2t2[:])


def get_program(reps=1):
    key = ("nc", reps)
    if key not in _CACHE:
        _CACHE[key] = _build_program(reps)
    return _CACHE[key]


def _to3d(mat512, cols, dtype):
    """(512, cols) feature-major -> (128, KT, cols) k-tile-major."""
    return np.ascontiguousarray(
        np.asarray(mat512, np.float32).reshape(KT, 128, cols)
        .transpose(1, 0, 2)).astype(dtype)


def make_in_maps(bef_feat, sent_feat, aft_feat, masks,
                 conv1_w, conv1_b, in_proj_w, out_proj_w, conv2_w, conv2_b,
                 q_w, q_b, k_w, k_b, logit_scale):
    bef_feat = np.asarray(bef_feat, np.float32)
    sent_feat = np.asarray(sent_feat, np.float32)
    aft_feat = np.asarray(aft_feat, np.float32)
    masks = np.asarray(masks, np.float32)
    conv1_w = np.asarray(conv1_w, np.float32)
    in_proj_w = np.asarray(in_proj_w, np.float32)
    out_proj_w = np.asarray(out_proj_w, np.float32)
    conv2_w = np.asarray(conv2_w, np.float32)
    conv2_b = np.asarray(conv2_b, np.float32)
    q_w = np.asarray(q_w, np.float32)
    q_b = np.asarray(q_b, np.float32)
    k_w = np.asarray(k_w, np.float32)
    k_b = np.asarray(k_b, np.float32)

    sent_mean = (sent_feat * masks[:, :, None]).mean(axis=1)       # (B, D)
    # v1 = W1a @ bef + txt, txt_a = conv1_w[:,D:] @ sent_mean_a + conv1_b
    W1a = conv1_w[:, :D]                                            # (D, D)
    txt = sent_mean @ conv1_w[:, D:].T + np.asarray(conv1_b, np.float32)

    Wq, Wk, Wv = (in_proj_w[0:D], in_proj_w[D:2 * D], in_proj_w[2 * D:3 * D])
    Wq_c, Wk_c, Wv_c = Wq @ W1a, Wk @ W1a, Wv @ W1a                 # (D, D)
    tq = txt @ Wq.T                                                 # (B, D)
    tk = txt @ Wk.T
    tv = txt @ Wv.T
    # combined out_proj -> conv2 -> q_w
    Wql_c = q_w @ conv2_w @ out_proj_w                              # (D, D)
    bql_c = q_b + conv2_b @ q_w.T                                   # (D,)

    aftT = _to3d(aft_feat.transpose(2, 0, 1).reshape(D, NKEY), NKEY, F8)

    amask = np.zeros((128, TQ * AL), np.float32)
    for t in range(TQ):
        for r in range(min(128, NQ - t * 128)):
            amask[r, t * AL + (t * 128 + r) // LV] = 1.0
    bmask = np.zeros((128, TK * B), np.float32)
    for t in range(TK):
        for r in range(128):
            bmask[r, t * B + (t * 128 + r) // LV] = 1.0

    wmats = {
        "wq8": _to3d(Wq_c.T * SW_QKV, D, F8),
        "wk8": _to3d(Wk_c.T * SW_QKV, D, F8),
        "wv8": _to3d(Wv_c.T * SW_QKV, D, F8),
        "wql8": _to3d(Wql_c.T * (FP8_SQ / S_OT) * SW_QL, D, F8),
        "wkl8": _to3d(k_w.T * FP8_SK, D, F8),
    }

    def pcol(vec, scale):
        # (D,) bias -> (128, KT) partition-major f32
        return np.ascontiguousarray(
            (np.asarray(vec, np.float32) * scale).reshape(KT, 128).T
        ).astype(np.float32)

    bvecs = {
        "bqlc": pcol(bql_c, FP8_SQ),
        "bkl": pcol(k_b, FP8_SK),
    }

    def tcols(tmat, c):
        # (B, D) per-batch bias -> (128, KT*AL) partition-major, col a*KT+m
        tx = np.zeros((128, KT * AL), np.float32)
        for a in range(AL):
            tx[:, a * KT:(a + 1) * KT] = tmat[c * AL + a].reshape(KT, 128).T
        return tx

    in_maps = []
    for c in range(NCORES):
        sl = slice(c * AL, (c + 1) * AL)
        befT = _to3d(bef_feat[sl].transpose(2, 0, 1).reshape(D, NQ), NQ, F8)
        m = {"befT": befT, "aftT": aftT,
             "tq": tcols(tq, c), "tk": tcols(tk, c),
             "tv16": (tv[sl] * SW_QKV).reshape(1, AL, D).astype(BF16),
             "amask": amask.astype(BF16), "bmask": bmask.astype(BF16)}
        m.update(wmats)
        m.update(bvecs)
        in_maps.append(m)
    return in_maps


def finish(outs, logit_scale):
    """outs: list of 8 per-core (2*AL, B) arrays -> scalar loss."""
    t2v = np.zeros((B, B), np.float64)
    v2t = np.zeros((B, B), np.float64)
    for c in range(NCORES):
        o = np.asarray(outs[c], np.float64)
        t2v[c * AL:(c + 1) * AL, :] = o[0:AL]
        v2t[c * AL:(c + 1) * AL, :] = o[AL:2 * AL]
    S = 0.5 * (t2v + v2t) * np.exp(np.float64(np.asarray(logit_scale)))

    def ce(m):
        lse = np.log(np.sum(np.exp(m - m.max(axis=1, keepdims=True)), axis=1)) \
            + m.max(axis=1)
        return -np.mean(np.diag(m) - lse)

    loss = 0.5 * (ce(S) + ce(S.T))
    return np.float32(loss)


def kernel(**inputs):
    from concourse.bass_utils import run_bass_kernel_spmd

    nc = get_program()
    in_maps = make_in_maps(**inputs)
    res = run_bass_kernel_spmd(nc, in_maps, core_ids=list(range(NCORES)))
    outs = [res.results[c]["out"] for c in range(NCORES)]
    return finish(outs, inputs["logit_scale"])


# revision 10
# speedup vs baseline: 2.9836x; 2.9836x over previous
"""Trainium2 Bass kernel for the cross-batch retrieval contrastive loss.

Pipeline per batch b (reference semantics):
  sent_mean = mean(sent_feat * masks)                     (host)
  v1   = conv1([bef^T; broadcast sent_mean])              -> (196, 512)
  MHA over 196 positions, out_proj                        -> (196, 512)
  mod  = conv2(o)                                         -> (196, 512)
  ql   = mod @ q_w^T + q_b ; kl = aft @ k_w^T + k_b       -> (196, 512)
  logits[a,b,l,m] = ql[a,l,:] . kl[b,m,:]   (head split is a no-op)
  t2v[a,b] = mean_l max_m ; v2t[a,b] = mean_m max_l
  loss = symmetric InfoNCE on S = 0.5*(t2v+v2t)*exp(logit_scale)  (host, 32x32)

Sharding: data-parallel over the query-batch axis 'a' (4 batches/core x 8
cores). Each core computes kl for all 32 key batches (replicated), its own
front-end, and both orientations of every (a, b) logits tile so that the
max over l and the max over m are both free-axis DVE reductions.

Performance structure: the kernel is DVE-bound (the 210 max-reductions over
fp32 PSUM run at 1 elem/cycle/lane with a 120-cycle PSUM access penalty per
instruction; no DVE fast mode applies to PSUM reads).  Design choices:
- every linear chain with no nonlinearity is folded on the host:
  q/k/v come straight from bef (per-batch text term as bias), and
  out_proj -> conv2 -> q_w is ONE matrix; weights pre-scaled so fp8
  entries sit in the format's comfortable range, with the inverse scale
  applied on the PSUM->SBUF ACT copy;
- logits matmuls write 2-bank PSUM tiles [128, 2, 512] so each DVE
  max-reduce covers 4 b-groups ([qn, 2, 2, 196] -> [qn, 4]), amortizing
  the PSUM access penalty;
- a tunable fraction of windows is instead copied by ACT to bf16 SBUF
  and scanned by DVE tensor_scalar accum-max at 4x (111ns/196 vs 1x from
  PSUM), balancing ACT vs DVE occupancy;
- softmax: all 8 heads' exp-colsums land in one [8, 196] psum tile (one
  matmul per (head, m-tile) with a row-offset output), ONE reciprocal per
  batch covers all heads, and a [2,128]-selector matmul broadcasts the
  per-head normalizers to the 128 partitions of each head pair;
- final 32x32 InfoNCE on the host in float64 (tiny).
"""

import numpy as np
import ml_dtypes

B, LV, LT, D, H = 32, 196, 40, 512, 8
NCORES = 8
AL = B // NCORES          # query batches per core
KT = D // 128             # 128-row feature tiles per 512-dim tensor
LSPLIT = [(0, 128), (128, 68)]   # 196 = 128 + 68
NQ = AL * LV              # 784 query position-rows per core
NKEY = B * LV             # 6272 key position-rows
TQ = (NQ + 127) // 128    # 7 stationary tiles over query rows
TK = NKEY // 128          # 49 stationary tiles over key rows
W2 = 2 * LV               # batch-pair moving width
W2P = 400                 # fe fp8 tile stride (16B-aligned for DoubleRow)
BF16 = ml_dtypes.bfloat16
F8 = ml_dtypes.float8_e4m3fn

# fp8 scale plan:
#   bef, q, k: x1   v(pos): x1   ot: x32 (via the 1/32 colsum)
#   ql: x128  kl: x8
# Combined weights are additionally pre-scaled by SW_* on the host so the
# fp8 entries are O(0.1); the ACT copy applies 1/SW_*.
FP8_SQ = 128.0
FP8_SK = 8.0
S_OT = 32.0
SW_QKV = 16.0             # host pre-scale on wq8/wk8/wv8 (copy scale 1/16)
SW_QL = 8.0               # extra pre-scale on the combined ql weight

# fraction control for ACT-staged windows: stage every STAGE_MOD-th
# double-window group (0 disables staging)
STAGE_O1 = 0  # stage o1 group when (group_idx % STAGE_O1) == 0
STAGE_O2 = 0  # o2 staging off initially

_CACHE = {}


def _build_program(reps=1):
    from contextlib import ExitStack
    import concourse.bacc as bacc
    import concourse.tile as tile
    from concourse import mybir

    f32 = mybir.dt.float32
    bf = mybir.dt.bfloat16
    f8 = mybir.dt.float8e4

    nc = bacc.Bacc("TRN2", target_bir_lowering=False, debug=False,
                   num_devices=NCORES)

    d = {
        "befT": nc.dram_tensor("befT", [128, KT, NQ], f8,
                               kind="ExternalInput").ap(),
        "aftT": nc.dram_tensor("aftT", [128, KT, NKEY], f8,
                               kind="ExternalInput").ap(),
        # per-batch per-partition bias terms (true units): q/k text bias
        "tq": nc.dram_tensor("tq", [128, KT * AL], f32,
                             kind="ExternalInput").ap(),
        "tk": nc.dram_tensor("tk", [128, KT * AL], f32,
                             kind="ExternalInput").ap(),
        # v text contribution x16, row-major on one partition: [1, AL, D]
        "tv16": nc.dram_tensor("tv16", [1, AL, D], bf,
                               kind="ExternalInput").ap(),
        "bqlc": nc.dram_tensor("bqlc", [128, KT], f32,
                               kind="ExternalInput").ap(),
        "bkl": nc.dram_tensor("bkl", [128, KT], f32,
                              kind="ExternalInput").ap(),
        "amask": nc.dram_tensor("amask", [128, TQ * AL], bf,
                                kind="ExternalInput").ap(),
        "bmask": nc.dram_tensor("bmask", [128, TK * B], bf,
                                kind="ExternalInput").ap(),
        "out": nc.dram_tensor("out", [2 * AL, B], f32, kind="ExternalOutput").ap(),
    }
    for n in ["wq8", "wk8", "wv8", "wql8", "wkl8"]:
        d[n] = nc.dram_tensor(n, [128, KT, D], f8, kind="ExternalInput").ap()

    with tile.TileContext(nc) as tc, ExitStack() as ctx:
        const = ctx.enter_context(tc.tile_pool(name="const", bufs=1))
        big = ctx.enter_context(tc.tile_pool(name="big", bufs=1))

        for _rep in range(reps):
            _kernel_body(nc, tc, mybir, const, big, d)

    nc.compile()
    return nc


def _kernel_body(nc, tc, mybir, const, big, d):
    from contextlib import ExitStack

    f32 = mybir.dt.float32
    bf = mybir.dt.bfloat16
    f8 = mybir.dt.float8e4
    AX = mybir.AxisListType.X
    MAX = mybir.AluOpType.max
    MULT = mybir.AluOpType.mult
    EXP = mybir.ActivationFunctionType.Exp
    IDENT = mybir.ActivationFunctionType.Identity
    DR = mybir.MatmulPerfMode.DoubleRow

    # ---- constants / weights into SBUF ----
    ones = const.tile([1, 128], bf, name="ones", tag="ones")
    nc.vector.memset(ones[:], 1.0)
    ones32 = const.tile([128, 1], bf, name="ones32", tag="ones32")
    nc.vector.memset(ones32[:], 1.0 / S_OT)

    sb_in = {}
    for n, dt_ in [("tq", f32), ("tk", f32), ("tv16", bf), ("bqlc", f32),
                   ("bkl", f32), ("amask", bf), ("bmask", bf)]:
        shape = list(d[n].tensor.shape)
        sb_in[n] = const.tile(shape, dt_, name=f"{n}_sb", tag=f"{n}_sb")
        nc.sync.dma_start(out=sb_in[n][:], in_=d[n][:])
    w = {}
    for n in ["wq8", "wk8", "wv8", "wql8", "wkl8"]:
        w[n] = const.tile([128, KT, D], f8, name=f"{n}_sb", tag=f"{n}_sb")
        nc.sync.dma_start(out=w[n][:], in_=d[n][:, :, :])

    befT = big.tile([128, KT, NQ], f8, name="bef8", tag="bef8")
    nc.sync.dma_start(out=befT[:], in_=d["befT"][:, :, :])
    aft = big.tile([128, KT, NKEY], f8, name="aft8", tag="aft8")
    nc.sync.dma_start(out=aft[:], in_=d["aftT"][:, :, :])
    klT = big.tile([128, KT, NKEY], f8, name="klT8", tag="klT8")
    qlT = big.tile([128, KT, NQ], f8, name="qlT8", tag="qlT8")
    cm = big.tile([128, TK, AL], bf, name="cm", tag="cm")
    rms = [big.tile([128, 4 * (NKEY // W2 // 2)], bf, name=f"rm{t}",
                    tag=f"rm{t}") for t in range(TQ)]

    # ================= front-end + klT phase =================
    with ExitStack() as fes:
        fe = fes.enter_context(tc.tile_pool(name="fe", bufs=2))
        ps = fes.enter_context(tc.tile_pool(name="ps", bufs=3, space="PSUM"))
        psb = fes.enter_context(tc.tile_pool(name="psb", bufs=3, space="PSUM"))
        ps2 = fes.enter_context(tc.tile_pool(name="ps2", bufs=2, space="PSUM"))

        def proj(dst, dst_col, src, src_col, wname, n, scale=1.0, bname=None,
                 txt_a=None, pool=None):
            """dst[:, m, dst_col:+n] (fp8) = fp8-DR W^T x src[:, :, src_col:+n],
            scale and bias applied on the PSUM->SBUF ACT copy."""
            pool = pool or ps
            for m in range(KT):
                p = pool.tile([128, 512], f32, name="p_proj", tag=pool.name)
                for j in range(KT // 2):
                    nc.tensor.matmul(
                        p[:, 0:n], lhsT=w[wname][:, 2 * j:2 * j + 2,
                                                 m * 128:(m + 1) * 128],
                        rhs=src[:, 2 * j:2 * j + 2, src_col:src_col + n],
                        start=(j == 0), stop=(j == KT // 2 - 1), perf_mode=DR)
                out_ap = dst[:, m, dst_col:dst_col + n]
                if txt_a is not None:
                    # per-batch text contribution as the copy's bias
                    for ab in range(2):
                        a = txt_a[0] + ab
                        nc.scalar.activation(
                            out_ap[:, ab * LV:(ab + 1) * LV],
                            p[:, ab * LV:(ab + 1) * LV], IDENT, scale=scale,
                            bias=sb_in[txt_a[1]][:, a * KT + m: a * KT + m + 1])
                elif bname is not None:
                    nc.scalar.activation(out_ap, p[:, 0:n], IDENT, scale=scale,
                                         bias=sb_in[bname][:, m:m + 1])
                else:
                    nc.scalar.activation(out_ap, p[:, 0:n], IDENT, scale=scale)

        # ---- front-end for the 4 local query batches (pairs) ----
        for apair in range(AL // 2):
            pc = apair * W2

            qt = fe.tile([128, KT, W2P], f8, name="qt", tag="qt")
            kt = fe.tile([128, KT, W2P], f8, name="kt", tag="kt")
            proj(qt, 0, befT, pc, "wq8", W2, scale=1.0 / SW_QKV,
                 txt_a=(apair * 2, "tq"))
            proj(kt, 0, befT, pc, "wk8", W2, scale=1.0 / SW_QKV,
                 txt_a=(apair * 2, "tk"))

            ot = fe.tile([128, KT, W2P], f8, name="ot", tag="ot")
            for ab in range(2):
                a = apair * 2 + ab
                ac = ab * LV
                # v position-major: (196, 512) as two row tiles (bf16)
                vpos = []
                for lt, (l0, ln) in enumerate(LSPLIT):
                    p5 = ps.tile([128, D], f32, name="p_vpos", tag="ps")
                    for j in range(KT // 2):
                        nc.tensor.matmul(
                            p5[0:ln, :],
                            lhsT=befT[:, 2 * j:2 * j + 2,
                                      pc + ac + l0:pc + ac + l0 + ln],
                            rhs=w["wv8"][:, 2 * j:2 * j + 2, :],
                            start=(j == 0), stop=False, perf_mode=DR)
                    # + 16x text contribution as a rank-1 update
                    nc.tensor.matmul(
                        p5[0:ln, :], lhsT=ones[0:1, 0:ln],
                        rhs=sb_in["tv16"][0:1, a, :], start=False, stop=True)
                    t = fe.tile([128, D], bf, name=f"vpos_{lt}",
                                tag=f"vpos_{lt}")
                    nc.scalar.activation(t[0:ln, :], p5[0:ln, :], IDENT,
                                         scale=1.0 / SW_QKV)
                    vpos.append(t)

                # attention; exp-scores per head, colsums for each HEAD PAIR
                # packed into one [1, 392] psum tile (both halves base-0),
                # one reciprocal per pair covers both heads
                eTs = {}
                rzbs = {}
                for kt2 in range(KT):
                    pz = ps2.tile([1, W2], f32, name="p_z", tag="ps2")
                    for hh in range(2):
                        h = kt2 * 2 + hh
                        off = 64 * hh
                        for mt, (m0, mn) in enumerate(LSPLIT):
                            psc = psb.tile([128, LV], f32, name="p_sc",
                                           tag="psb")
                            nc.tensor.matmul(
                                psc[0:mn, :],
                                lhsT=kt[off:off + 64, kt2,
                                        ac + m0:ac + m0 + mn],
                                rhs=qt[off:off + 64, kt2, ac:ac + LV],
                                start=True, stop=True)
                            e = fe.tile([128, LV], bf, name=f"eT_{h}_{mt}",
                                        tag=f"eT_{h}_{mt}")
                            nc.scalar.activation(e[0:mn, :], psc[0:mn, :],
                                                 EXP, scale=0.125)
                            eTs[(h, mt)] = e
                        for mt, (m0, mn) in enumerate(LSPLIT):
                            nc.tensor.matmul(
                                pz[0:1, hh * LV:(hh + 1) * LV],
                                lhsT=ones32[0:mn, 0:1],
                                rhs=eTs[(h, mt)][0:mn, :], start=(mt == 0),
                                stop=(mt == 1))
                    rzb = fe.tile([1, W2], bf, name=f"rzb_{kt2}",
                                  tag=f"rzb_{kt2}")
                    with nc.allow_low_precision(
                            reason="softmax normalizer consumed in fp8"):
                        nc.vector.reciprocal(rzb[:], pz[:])
                    rzbs[kt2] = rzb

                for kt2 in range(KT):
                    po = ps.tile([128, LV], f32, name="p_o", tag="ps")
                    pzb = psb.tile([128, LV], f32, name="p_zb", tag="psb")
                    for hh in range(2):
                        nc.tensor.matmul(
                            pzb[64 * hh:64 * hh + 64, :],
                            lhsT=ones[0:1, 0:64],
                            rhs=rzbs[kt2][0:1, hh * LV:(hh + 1) * LV],
                            start=True, stop=True)
                        h = kt2 * 2 + hh
                        off = 64 * hh
                        for mt, (m0, mn) in enumerate(LSPLIT):
                            nc.tensor.matmul(
                                po[off:off + 64, :],
                                lhsT=vpos[mt][0:mn, h * 64:(h + 1) * 64],
                                rhs=eTs[(h, mt)][0:mn, :], start=(mt == 0),
                                stop=(mt == 1))
                    zb = fe.tile([128, LV], f32, name="zb", tag="zb")
                    nc.scalar.copy(zb[:], pzb[:])
                    nc.vector.tensor_mul(ot[:, kt2, ac:ac + LV], po[:], zb[:])

            # combined out_proj -> conv2 -> q_w projection
            proj(qlT, pc, ot, 0, "wql8", W2, scale=1.0 / SW_QL, bname="bqlc")

        # ---- klT for all 32 key batches (fp8 DR) ----
        for w0 in range(0, NKEY, 512):
            proj(klT, w0, aft, w0, "wkl8", min(512, NKEY - w0), bname="bkl",
             pool=psb)

    # ================= logits phase: both orientations =================
    # o1: 784 query rows in 7 stationary tiles, rhs = key pairs; 2-bank psum
    # groups of 4 b's with a single [qn,2,2,196]->[qn,4] DVE reduce (or an
    # ACT-staged bf16 copy + 4x tensor_scalar accum-max scans).
    # o2: 6272 key rows in 49 stationary tiles, rhs = query pairs, same
    # grouping over pairs of stationary tiles.
    NW = NKEY // W2           # 16 windows per o1 stationary tile
    with ExitStack() as ls:
        st = ls.enter_context(tc.tile_pool(name="st", bufs=3))
        ps = ls.enter_context(tc.tile_pool(name="ps", bufs=2, space="PSUM"))
        psb = ls.enter_context(tc.tile_pool(name="psb", bufs=2, space="PSUM"))

        def scan(p, outs, staged):
            """p: [128, 2, 512] psum, 2x2 groups of 196; outs: list of 4
            [qn, 1]-ish APs (or one [qn, 4] AP as outs=[ap]) for maxes."""
            if not staged:
                nc.vector.tensor_reduce(
                    outs[0], p[:, :, 0:W2].rearrange(
                        "p k (two n) -> p k two n", two=2),
                    axis=AX, op=MAX)
            else:
                qn = p.shape[0]
                sg = st.tile([128, 2, W2], bf, name="stage", tag="st")
                nc.scalar.activation(sg[0:qn], p[:, :, 0:W2], IDENT)
                for i in range(4):
                    nc.vector.tensor_scalar(
                        out=sg[0:qn, i // 2, (i % 2) * LV:(i % 2 + 1) * LV],
                        in0=sg[0:qn, i // 2, (i % 2) * LV:(i % 2 + 1) * LV],
                        scalar1=1.0, scalar2=None, op0=MULT, op1=MAX,
                        accum_out=outs[1][i])

        def o1_stream():
            gi = 0
            for t in range(TQ):
                q0 = t * 128
                qn = min(128, NQ - q0)
                rm = rms[t]
                for g in range(NW // 2):
                    p = ps.tile([128, 2, 512], f32, name="p_lg", tag="ps")
                    for k in range(2):
                        wi = (2 * g + k) * W2
                        for j in range(KT // 2):
                            nc.tensor.matmul(
                                p[0:qn, k, 0:W2],
                                lhsT=qlT[:, 2 * j:2 * j + 2, q0:q0 + qn],
                                rhs=klT[:, 2 * j:2 * j + 2, wi:wi + W2],
                                start=(j == 0), stop=(j == KT // 2 - 1),
                                perf_mode=DR)
                    staged = STAGE_O1 and (gi % STAGE_O1 == 0)
                    scan(p[0:qn], [rm[0:qn, 4 * g:4 * g + 4],
                                   [rm[0:qn, 4 * g + i:4 * g + i + 1]
                                    for i in range(4)]], staged)
                    gi += 1
                    yield

        def o2_stream():
            gi = 0
            for apair in range(AL // 2):
                pc = apair * W2
                for g0 in range(0, TK - 1, 2):
                    p = psb.tile([128, 2, 512], f32, name="p_lg2", tag="psb")
                    for k in range(2):
                        t = g0 + k
                        for j in range(KT // 2):
                            nc.tensor.matmul(
                                p[:, k, 0:W2],
                                lhsT=klT[:, 2 * j:2 * j + 2,
                                         t * 128:(t + 1) * 128],
                                rhs=qlT[:, 2 * j:2 * j + 2, pc:pc + W2],
                                start=(j == 0), stop=(j == KT // 2 - 1),
                                perf_mode=DR)
                    staged = STAGE_O2 and (gi % STAGE_O2 == 0)
                    scan(p, [cm[:, g0:g0 + 2, 2 * apair:2 * apair + 2],
                             [cm[:, g0 + i // 2,
                                 2 * apair + i % 2:2 * apair + i % 2 + 1]
                              for i in range(4)]], staged)
                    gi += 1
                    yield
                # odd tail tile (TK = 49)
                t = TK - 1
                p = psb.tile([128, 2, 512], f32, name="p_lg2", tag="psb")
                for j in range(KT // 2):
                    nc.tensor.matmul(
                        p[:, 0, 0:W2],
                        lhsT=klT[:, 2 * j:2 * j + 2, t * 128:(t + 1) * 128],
                        rhs=qlT[:, 2 * j:2 * j + 2, pc:pc + W2],
                        start=(j == 0), stop=(j == KT // 2 - 1), perf_mode=DR)
                nc.vector.tensor_reduce(
                    cm[:, t, 2 * apair:2 * apair + 2],
                    p[:, 0, 0:W2].rearrange("p (two n) -> p two n", two=2),
                    axis=AX, op=MAX)
                yield

        s1, s2 = o1_stream(), o2_stream()
        done1 = done2 = False
        while not (done1 and done2):
            if not done1:
                done1 = next(s1, "END") == "END"
            if not done2:
                done2 = next(s2, "END") == "END"

        # ---- attribution + output ----
        OSCALE = 1.0 / (LV * FP8_SQ * FP8_SK)
        pacc = ps.tile([128, 2, 512], f32, name="pacc", tag="ps")
        for t in range(TQ):
            qn = min(128, NQ - t * 128)
            nc.tensor.matmul(pacc[0:AL, 0, 0:B],
                             lhsT=sb_in["amask"][0:qn, t * AL:(t + 1) * AL],
                             rhs=rms[t][0:qn, :],
                             start=(t == 0), stop=(t == TQ - 1))
        t2v_sb = st.tile([AL, B], f32, name="t2v_sb", tag="t2v_sb")
        nc.scalar.mul(t2v_sb[:], pacc[0:AL, 0, 0:B], OSCALE)
        nc.sync.dma_start(out=d["out"][0:AL, :], in_=t2v_sb[:])

        pv2 = psb.tile([128, 2, 512], f32, name="pv2", tag="psb")
        for t in range(TK):
            nc.tensor.matmul(pv2[0:AL, 0, 0:B], lhsT=cm[:, t, :],
                             rhs=sb_in["bmask"][:, t * B:(t + 1) * B],
                             start=(t == 0), stop=(t == TK - 1))
        v2t2 = st.tile([AL, B], f32, name="v2t2", tag="v2t2")
        nc.scalar.mul(v2t2[:], pv2[0:AL, 0, 0:B], OSCALE)
        nc.sync.dma_start(out=d["out"][AL:2 * AL, :], in_=v2t2[:])


def get_program(reps=1):
    key = ("nc", reps)
    if key not in _CACHE:
        _CACHE[key] = _build_program(reps)
    return _CACHE[key]


def _to3d(mat512, cols, dtype):
    """(512, cols) feature-major -> (128, KT, cols) k-tile-major."""
    return np.ascontiguousarray(
        np.asarray(mat512, np.float32).reshape(KT, 128, cols)
        .transpose(1, 0, 2)).astype(dtype)


def make_in_maps(bef_feat, sent_feat, aft_feat, masks,
                 conv1_w, conv1_b, in_proj_w, out_proj_w, conv2_w, conv2_b,
                 q_w, q_b, k_w, k_b, logit_scale):
    bef_feat = np.asarray(bef_feat, np.float32)
    sent_feat = np.asarray(sent_feat, np.float32)
    aft_feat = np.asarray(aft_feat, np.float32)
    masks = np.asarray(masks, np.float32)
    conv1_w = np.asarray(conv1_w, np.float32)
    in_proj_w = np.asarray(in_proj_w, np.float32)
    out_proj_w = np.asarray(out_proj_w, np.float32)
    conv2_w = np.asarray(conv2_w, np.float32)
    conv2_b = np.asarray(conv2_b, np.float32)
    q_w = np.asarray(q_w, np.float32)
    q_b = np.asarray(q_b, np.float32)
    k_w = np.asarray(k_w, np.float32)
    k_b = np.asarray(k_b, np.float32)

    sent_mean = (sent_feat * masks[:, :, None]).mean(axis=1)       # (B, D)
    # v1 = W1a @ bef + txt, txt_a = conv1_w[:,D:] @ sent_mean_a + conv1_b
    W1a = conv1_w[:, :D]                                            # (D, D)
    txt = sent_mean @ conv1_w[:, D:].T + np.asarray(conv1_b, np.float32)

    Wq, Wk, Wv = (in_proj_w[0:D], in_proj_w[D:2 * D], in_proj_w[2 * D:3 * D])
    Wq_c, Wk_c, Wv_c = Wq @ W1a, Wk @ W1a, Wv @ W1a                 # (D, D)
    tq = txt @ Wq.T                                                 # (B, D)
    tk = txt @ Wk.T
    tv = txt @ Wv.T
    # combined out_proj -> conv2 -> q_w
    Wql_c = q_w @ conv2_w @ out_proj_w                              # (D, D)
    bql_c = q_b + conv2_b @ q_w.T                                   # (D,)

    aftT = _to3d(aft_feat.transpose(2, 0, 1).reshape(D, NKEY), NKEY, F8)

    amask = np.zeros((128, TQ * AL), np.float32)
    for t in range(TQ):
        for r in range(min(128, NQ - t * 128)):
            amask[r, t * AL + (t * 128 + r) // LV] = 1.0
    bmask = np.zeros((128, TK * B), np.float32)
    for t in range(TK):
        for r in range(128):
            bmask[r, t * B + (t * 128 + r) // LV] = 1.0

    wmats = {
        "wq8": _to3d(Wq_c.T * SW_QKV, D, F8),
        "wk8": _to3d(Wk_c.T * SW_QKV, D, F8),
        "wv8": _to3d(Wv_c.T * SW_QKV, D, F8),
        "wql8": _to3d(Wql_c.T * (FP8_SQ / S_OT) * SW_QL, D, F8),
        "wkl8": _to3d(k_w.T * FP8_SK, D, F8),
    }

    def pcol(vec, scale):
        # (D,) bias -> (128, KT) partition-major f32
        return np.ascontiguousarray(
            (np.asarray(vec, np.float32) * scale).reshape(KT, 128).T
        ).astype(np.float32)

    bvecs = {
        "bqlc": pcol(bql_c, FP8_SQ),
        "bkl": pcol(k_b, FP8_SK),
    }

    def tcols(tmat, c):
        # (B, D) per-batch bias -> (128, KT*AL) partition-major, col a*KT+m
        tx = np.zeros((128, KT * AL), np.float32)
        for a in range(AL):
            tx[:, a * KT:(a + 1) * KT] = tmat[c * AL + a].reshape(KT, 128).T
        return tx

    in_maps = []
    for c in range(NCORES):
        sl = slice(c * AL, (c + 1) * AL)
        befT = _to3d(bef_feat[sl].transpose(2, 0, 1).reshape(D, NQ), NQ, F8)
        m = {"befT": befT, "aftT": aftT,
             "tq": tcols(tq, c), "tk": tcols(tk, c),
             "tv16": (tv[sl] * SW_QKV).reshape(1, AL, D).astype(BF16),
             "amask": amask.astype(BF16), "bmask": bmask.astype(BF16)}
        m.update(wmats)
        m.update(bvecs)
        in_maps.append(m)
    return in_maps


def finish(outs, logit_scale):
    """outs: list of 8 per-core (2*AL, B) arrays -> scalar loss."""
    t2v = np.zeros((B, B), np.float64)
    v2t = np.zeros((B, B), np.float64)
    for c in range(NCORES):
        o = np.asarray(outs[c], np.float64)
        t2v[c * AL:(c + 1) * AL, :] = o[0:AL]
        v2t[c * AL:(c + 1) * AL, :] = o[AL:2 * AL]
    S = 0.5 * (t2v + v2t) * np.exp(np.float64(np.asarray(logit_scale)))

    def ce(m):
        lse = np.log(np.sum(np.exp(m - m.max(axis=1, keepdims=True)), axis=1)) \
            + m.max(axis=1)
        return -np.mean(np.diag(m) - lse)

    loss = 0.5 * (ce(S) + ce(S.T))
    return np.float32(loss)


def kernel(**inputs):
    from concourse.bass_utils import run_bass_kernel_spmd

    nc = get_program()
    in_maps = make_in_maps(**inputs)
    res = run_bass_kernel_spmd(nc, in_maps, core_ids=list(range(NCORES)))
    outs = [res.results[c]["out"] for c in range(NCORES)]
    return finish(outs, inputs["logit_scale"])
